# revision 23
# baseline (speedup 1.0000x reference)
"""Causal self-attention (B=4, T=2048, C=768, 12 heads) on 8 Trainium2 cores.

Sharding: core i handles batch b = i//2 and head-set s = i%2 (6 of 12 heads).
Each core computes x[b] @ W_attn slice -> 6 heads of causal attention -> a
partial projection (row-sharded W_proj).  The host sums the two partials per
batch and adds b_proj.

Device layout per core:
  - x^T [768, 2048] (host pre-transposed), f32r.
  - Q^T/K^T in [head_cols, T] "pair" layout [128, 2048] (head a on
    partitions 0-63, head b on 64-127); 1/sqrt(64) folded into W_q/b_q.
  - V' [2048, 6*65] natural layout with an all-ones column per head
    (zero weight column + bias 1.0), bf16: PV then yields both Y and the
    softmax denominator column.
  - S^T = K^T.T @ Q^T per (head, q-chunk 512, k-chunk 128), causal tiles
    only with per-diagonal shrink; exp on ScalarE out of PSUM into bf16
    pt tiles; the 128-wide diagonal square gets a 0/1 mask multiply (DVE).
  - PV in the *natural* direction: Y[128q, 65] += pt_slice.T @ V'_block,
    accumulated over k-chunks in PSUM.  Denominator lands in column 64,
    so normalization is a cheap per-partition reciprocal + tensor_scalar
    (no cross-partition broadcast).
  - Normalized Y (bf16) is transposed back to [feat, tok] via PE identity
    transposes, then the partial projection accumulates over the 3 pairs.
  - QKV / V' / projection matmuls are emitted as "fillers" interleaved
    into the attention loops so the PE queue never head-of-line blocks
    on the exp chain.
"""

from collections import deque

import numpy as np

import concourse.bass as bass
import concourse.mybir as mybir
import concourse.tile as tile
from concourse import bacc

B, T, C = 4, 2048, 768
NH, HD = 12, 64
N_CORES = 8
HPC = 6  # heads per core
P = 128
F32 = mybir.dt.float32
F32R = mybir.dt.float32r
BF16 = mybir.dt.bfloat16
QC_N = T // 512  # 4 q-chunks of 512
KC_N = T // P    # 16 k-chunks of 128
CKC = C // P     # 6 contraction chunks for the QKV projection


def build_program(n_iters: int = 1, pace: int = 3, **_compat):
    """Builds the SPMD program (identical on all cores; data differs)."""
    PACE = pace
    nc = bacc.Bacc(
        "TRN2",
        target_bir_lowering=False,
        debug=False,
        enable_asserts=False,
        num_devices=N_CORES,
    )
    d_xt = nc.dram_tensor("xt", [C, T], BF16, kind="ExternalInput").ap()
    d_wq = nc.dram_tensor("wq", [C, 384], BF16, kind="ExternalInput").ap()
    d_wk = nc.dram_tensor("wk", [C, 384], BF16, kind="ExternalInput").ap()
    d_wv = nc.dram_tensor("wv", [C, 390], BF16, kind="ExternalInput").ap()
    d_w2 = nc.dram_tensor("w2", [384, C], BF16, kind="ExternalInput").ap()
    d_bq = nc.dram_tensor("bq", [P, 3], F32, kind="ExternalInput").ap()
    d_bk = nc.dram_tensor("bk", [P, 3], F32, kind="ExternalInput").ap()
    d_bv = nc.dram_tensor("bv", [1, 390], BF16, kind="ExternalInput").ap()
    d_ones = nc.dram_tensor("ones", [1, P], BF16, kind="ExternalInput").ap()
    d_mask = nc.dram_tensor("masks", [P, P], BF16, kind="ExternalInput").ap()
    d_id = nc.dram_tensor("ident", [P, P], BF16, kind="ExternalInput").ap()
    d_out = nc.dram_tensor("out", [T, C], BF16, kind="ExternalOutput").ap()

    with tile.TileContext(nc) as tc:
        # PSUM budget (8 banks):
        #   tag "ps_S" [128,1024] x2 = 4 banks (S^T tiles; QKV/V'/proj psum)
        #   tag "y0"/"y1" [128,260] x2 each = 4 banks (Y accum; transposes)
        const_cm = tc.tile_pool(name="const", bufs=1)
        work_cm = tc.tile_pool(name="work", bufs=1)
        sb_cm = tc.tile_pool(name="sbw", bufs=2)
        ps_cm = tc.tile_pool(name="psum", bufs=1, space="PSUM")
        const = const_cm.__enter__()
        work = work_cm.__enter__()
        sbw = sb_cm.__enter__()
        psp = ps_cm.__enter__()

        def body(_i=None):
            wq_sb = [const.tile([P, 384], BF16, tag=f"wq{k}", name=f"wq{k}") for k in range(CKC)]
            wk_sb = [const.tile([P, 384], BF16, tag=f"wk{k}", name=f"wk{k}") for k in range(CKC)]
            wv_sb = [const.tile([P, 390], BF16, tag=f"wv{k}", name=f"wv{k}") for k in range(CKC)]
            w2_sb = [const.tile([P, C], BF16, tag=f"w2{p}", name=f"w2{p}") for p in range(3)]
            bq_sb = const.tile([P, 3], F32, tag="bq")
            bk_sb = const.tile([P, 3], F32, tag="bk")
            bv_sb = const.tile([1, 390], BF16, tag="bv")
            ones_sb = const.tile([1, P], BF16, tag="ones")
            mask_sb = const.tile([P, P], BF16, tag="masks")
            id_sb = const.tile([P, P], BF16, tag="ident")
            xt_sb = [work.tile([P, T], BF16, tag=f"xt{k}", name=f"xt{k}") for k in range(CKC)]
            qt_sb = [work.tile([P, T], F32R, tag=f"qt{p}", name=f"qtp{p}") for p in range(3)]
            kt_sb = [work.tile([P, T], F32R, tag=f"kt{p}", name=f"ktp{p}") for p in range(3)]
            v_sb = [work.tile([P, 390], BF16, tag=f"v{t}", name=f"v{t}") for t in range(KC_N)]
            ynT_sb = [work.tile([P, T], BF16, tag=f"ynT{p}", name=f"ynT{p}") for p in range(3)]

            # ---- loads, priority order ----
            # Each dma_start costs ~500ns of issue time on its engine's
            # sequencer, so use whole-tile transfers and round-robin the
            # triggers over queues that are idle at startup.
            _eng = [nc.sync, nc.gpsimd, nc.scalar]
            _ei = [0]

            def load(dst, src):
                _eng[_ei[0] % len(_eng)].dma_start(dst, src)
                _ei[0] += 1

            for k in range(CKC):
                load(xt_sb[k][:, 0:512], d_xt[k * P:(k + 1) * P, 0:512])
                load(wq_sb[k][:], d_wq[k * P:(k + 1) * P, :])
                load(wk_sb[k][:], d_wk[k * P:(k + 1) * P, :])
            load(bq_sb[:], d_bq[:])
            load(bk_sb[:], d_bk[:])
            for k in range(CKC):
                load(wv_sb[k][:], d_wv[k * P:(k + 1) * P, :])
            for k in range(CKC):
                load(xt_sb[k][:, 512:T], d_xt[k * P:(k + 1) * P, 512:T])
            load(bv_sb[:], d_bv[:])
            load(ones_sb[:], d_ones[:])
            load(mask_sb[:], d_mask[:])
            load(id_sb[:], d_id[:])
            for p in range(3):
                load(w2_sb[p][:], d_w2[p * P:(p + 1) * P, :])

            # Pre-load the ScalarE Exp table while QKV is still running: a
            # tiny dummy exp out of a const tile.
            warm = sbw.tile([P, 3], F32, tag="warm", bufs=1)
            nc.scalar.activation(
                warm[:], bq_sb[:], mybir.ActivationFunctionType.Exp
            )

            # ---- filler emission units ----
            def emit_qk(p, qc, w_sb, b_sb, o_sb):
                ps = psp.tile([P, 1024], F32, tag="ps_S", bufs=3, name=f"qk{p}{qc}")
                for k in range(CKC):
                    nc.tensor.matmul(
                        ps[:, 0:512],
                        lhsT=w_sb[k][:, p * P:(p + 1) * P],
                        rhs=xt_sb[k][:, qc * 512:(qc + 1) * 512],
                        start=(k == 0),
                        stop=(k == CKC - 1),
                    )
                nc.vector.tensor_scalar(
                    o_sb[p][:, qc * 512:(qc + 1) * 512],
                    ps[:, 0:512],
                    b_sb[:, p:p + 1],
                    None,
                    mybir.AluOpType.add,
                )

            def emit_v(t):
                ps = psp.tile([P, 1024], F32, tag="ps_S", bufs=3, name=f"vt{t}")
                for k in range(CKC):
                    nc.tensor.matmul(
                        ps[:, :390],
                        lhsT=xt_sb[k][:, t * P:(t + 1) * P],
                        rhs=wv_sb[k][:],
                        start=(k == 0),
                        stop=False,
                    )
                nc.tensor.matmul(
                    ps[:, :390], lhsT=ones_sb[:, :P], rhs=bv_sb[:],
                    start=False, stop=True,
                )
                nc.vector.tensor_copy(v_sb[t][:], ps[:, :390])

            def emit_proj(qb):
                po = psp.tile([P, 1024], F32, tag="ps_S", bufs=3, name=f"po{qb}")
                for (n0, nw) in ((0, 512), (512, 256)):
                    for pp in range(3):
                        nc.tensor.matmul(
                            po[:, n0:n0 + nw],
                            lhsT=ynT_sb[pp][:, qb * P:(qb + 1) * P],
                            rhs=w2_sb[pp][:, n0:n0 + nw],
                            start=(pp == 0),
                            stop=(pp == 2),
                        )
                ob = sbw.tile([P, C], BF16, tag="ob", bufs=3, name=f"ob{qb}")
                nc.vector.tensor_copy(ob[:, 0:512], po[:, 0:512])
                nc.vector.tensor_copy(ob[:, 512:768], po[:, 512:768])
                nc.sync.dma_start(d_out[qb * P:(qb + 1) * P, :], ob[:])

            def emit_pv(qc, p, yps, kc, pt, m):
                # One PSUM accumulation bracket per yps bank: start zeroes the
                # whole 2KB zero-region, so the four 65-col q-sub groups share
                # a single start (kc 0, j 0) / stop (last kc, j 3).
                n_kc = 4 * qc + 4
                for h2 in range(2):
                    ch = p * 2 + h2
                    for j in range(max(m, 0), 4):
                        nc.tensor.matmul(
                            yps[h2][:, j * 65:(j + 1) * 65],
                            lhsT=pt[:, h2 * 512 + j * 128:
                                    h2 * 512 + (j + 1) * 128],
                            rhs=v_sb[kc][:, ch * 65:(ch + 1) * 65],
                            start=(kc == 0 and j == 0),
                            stop=(kc == n_kc - 1 and j == 3),
                        )

            # QKV fillers in qc-major order: attention(qc, p) needs Q(p, qc)
            # and K(p, qc' <= qc), which is exactly the prefix through
            # (qc, p) in this ordering.  V' tiles for q-chunk qc slot in
            # before the qc+1 block.
            fillers = deque()
            done = {"qk": 0, "v": 4}
            for qc in range(QC_N):
                if qc:
                    for t in range(4 * qc, 4 * qc + 4):
                        fillers.append(("v", lambda t=t: emit_v(t)))
                for p in range(3):
                    for (w_sb, b_sb, o_sb) in ((wq_sb, bq_sb, qt_sb),
                                               (wk_sb, bk_sb, kt_sb)):
                        fillers.append(
                            ("qk", lambda p=p, qc=qc, w=w_sb, b=b_sb, o=o_sb:
                             emit_qk(p, qc, w, b, o)))

            gkc = [0]  # global kc step counter, for filler pacing

            def pop_one():
                if fillers:
                    key, fn = fillers.popleft()
                    fn()
                    done[key] = done.get(key, 0) + 1

            def pop_paced():
                # ~one filler (~1.2us PE) per pace kc steps: matches the PE
                # slack left by the Act-limited exp chain without flooding
                # the PE queue ahead of attention-critical matmuls.
                gkc[0] += 1
                if gkc[0] % PACE == 0:
                    pop_one()

            def drain_until(check):
                while fillers and not check():
                    pop_one()

            def drain_all():
                while fillers:
                    pop_one()

            # ---- warmup: Q/K(p0, qc0) + first V' tiles ----
            drain_until(lambda: done["qk"] >= 2)
            for t in range(4):
                emit_v(t)

            # ---- main attention loop ----
            for qc in range(QC_N):
                n_kc = 4 * qc + 4
                # make sure V' tiles for this qc's k range are emitted
                drain_until(lambda: done["v"] >= n_kc)
                for p in range(3):
                    # Q(p, qc) / K(p, <=qc) must be emitted before use
                    drain_until(lambda: done["qk"] >= 2 * (3 * qc + p + 1))
                    yps = [psp.tile([P, 260], F32, tag=f"y{h2}", bufs=1,
                                    name=f"yp{qc}{p}{h2}") for h2 in range(2)]
                    pending = deque()  # (kc, pt, m) awaiting PV
                    for kc in range(n_kc):
                        m = kc - 4 * qc
                        s0s = 0 if m < 1 else (128 if m == 1 else 256)
                        s0e = 0 if m < 1 else 128 * m
                        ss = psp.tile([P, 1024], F32, tag="ps_S", bufs=3,
                                      name=f"ss{qc}{p}{kc}")
                        for h2 in range(2):
                            pb = 64 * h2
                            nc.tensor.matmul(
                                ss[:, h2 * 512 + s0s:(h2 + 1) * 512],
                                lhsT=kt_sb[p][pb:pb + 64, kc * P:(kc + 1) * P],
                                rhs=qt_sb[p][pb:pb + 64,
                                             qc * 512 + s0s:(qc + 1) * 512],
                                start=True,
                                stop=True,
                            )
                        pt = sbw.tile([P, 1024], BF16, tag="pt", bufs=4,
                                      name=f"pt{qc}{p}{kc}")
                        if s0e:
                            ss_r = ss.rearrange("p (h c) -> p h c", h=2)
                            pt_r = pt.rearrange("p (h c) -> p h c", h=2)
                            nc.scalar.activation(
                                pt_r[:, :, s0e:], ss_r[:, :, s0e:],
                                mybir.ActivationFunctionType.Exp,
                            )
                        else:
                            nc.scalar.activation(
                                pt[:], ss[:], mybir.ActivationFunctionType.Exp
                            )
                        if m >= 0:
                            for h2 in range(2):
                                c0 = h2 * 512 + 128 * m
                                nc.vector.tensor_tensor(
                                    pt[:, c0:c0 + 128],
                                    pt[:, c0:c0 + 128],
                                    mask_sb[:],
                                    mybir.AluOpType.mult,
                                )
                        # software pipeline, depth 2: PE stays two S tiles
                        # ahead of the exp-dependent PV matmuls
                        pending.append((kc, pt, m))
                        if len(pending) > 2:
                            emit_pv(qc, p, yps, *pending.popleft())
                        pop_paced()
                    while pending:
                        emit_pv(qc, p, yps, *pending.popleft())

                    # ---- normalize + transpose back to [feat, tok] ----
                    for h2 in range(2):
                        ypr = yps[h2].rearrange("p (q c) -> p q c", q=4)
                        recip = sbw.tile([P, 4], F32, tag="recip", bufs=2,
                                         name=f"rc{qc}{p}{h2}")
                        with nc.allow_low_precision("f32r is fp32 storage"):
                            nc.vector.reciprocal(recip[:], ypr[:, :, 64])
                        yns = []
                        for j in range(4):
                            yn = sbw.tile([P, 64], BF16, tag="yn", bufs=8,
                                          name=f"yn{qc}{p}{h2}{j}")
                            nc.vector.tensor_scalar(
                                yn[:], ypr[:, j, 0:64], recip[:, j:j + 1],
                                None, mybir.AluOpType.mult,
                            )
                            yns.append(yn)
                        # allocated after the yps reads above: same slot (tag),
                        # so the first write waits for them.  One accumulation
                        # bracket for the bank; disjoint regions add onto zero.
                        tp = psp.tile([64, 512], BF16, tag=f"y{h2}", bufs=1,
                                      name=f"tp{qc}{p}{h2}")
                        for j in range(4):
                            nc.tensor.matmul(
                                tp[:, j * 128:(j + 1) * 128],
                                lhsT=yns[j][:], rhs=id_sb[:],
                                is_transpose=True,
                                start=(j == 0), stop=(j == 3),
                            )
                        nc.vector.tensor_copy(
                            ynT_sb[p][h2 * 64:(h2 + 1) * 64,
                                      qc * 512:(qc + 1) * 512],
                            tp[:],
                        )
                # ---- output projection for this q-chunk ----
                if qc < QC_N - 1:
                    for qb in range(4 * qc, 4 * qc + 4):
                        fillers.append(("proj", lambda qb=qb: emit_proj(qb)))
                else:
                    drain_all()
                    for qb in range(4 * qc, 4 * qc + 4):
                        emit_proj(qb)
            drain_all()

        if n_iters == 1:
            body()
        else:
            with tc.For_i(0, n_iters, 1) as _i:
                body(_i)

        for cm in (ps_cm, sb_cm, work_cm, const_cm):
            cm.__exit__(None, None, None)

    nc.compile()
    return nc


def shard_inputs(x, W_attn, b_attn, W_proj, b_proj, **_compat):
    """Builds the 8 per-core input maps (all host-side numpy prep)."""
    import ml_dtypes

    x = np.asarray(x, dtype=np.float32)
    W_attn = np.asarray(W_attn, dtype=np.float32)
    b_attn = np.asarray(b_attn, dtype=np.float32)
    W_proj = np.asarray(W_proj, dtype=np.float32)
    scale = float(HD) ** -0.5

    kl = np.arange(P)[:, None]
    ql = np.arange(P)[None, :]
    mask_sq = (kl <= ql).astype(np.float32).astype(ml_dtypes.bfloat16)
    ident = np.eye(P, dtype=np.float32).astype(ml_dtypes.bfloat16)
    ones_row = np.ones((1, P), dtype=np.float32)

    in_maps = []
    for core in range(N_CORES):
        b = core // 2
        s = core % 2
        heads = [s * HPC + j for j in range(HPC)]
        xt = np.ascontiguousarray(x[b].T)  # [C, T]

        wq = np.empty((C, 384), np.float32)
        wk = np.empty((C, 384), np.float32)
        bq = np.empty((P, 3), np.float32)
        bk = np.empty((P, 3), np.float32)
        for p in range(3):
            for h2 in range(2):
                hh = heads[p * 2 + h2]
                cols = slice(hh * HD, (hh + 1) * HD)
                dst = slice(h2 * HD, (h2 + 1) * HD)
                wq[:, p * P + h2 * HD:p * P + (h2 + 1) * HD] = (
                    W_attn[:, cols] * scale
                )
                wk[:, p * P + h2 * HD:p * P + (h2 + 1) * HD] = (
                    W_attn[:, C + hh * HD:C + (hh + 1) * HD]
                )
                bq[dst, p] = b_attn[hh * HD:(hh + 1) * HD] * scale
                bk[dst, p] = b_attn[C + hh * HD:C + (hh + 1) * HD]

        wv = np.zeros((C, 390), np.float32)
        bv = np.zeros((1, 390), np.float32)
        for ch in range(HPC):
            hh = heads[ch]
            wv[:, ch * 65:ch * 65 + HD] = W_attn[:, 2 * C + hh * HD:2 * C + (hh + 1) * HD]
            bv[0, ch * 65:ch * 65 + HD] = b_attn[2 * C + hh * HD:2 * C + (hh + 1) * HD]
            bv[0, ch * 65 + HD] = 1.0

        w2 = np.empty((384, C), np.float32)
        for p in range(3):
            for h2 in range(2):
                hh = heads[p * 2 + h2]
                w2[p * P + h2 * HD:p * P + (h2 + 1) * HD, :] = (
                    W_proj[hh * HD:(hh + 1) * HD, :]
                )

        in_maps.append({
            "xt": xt.astype(ml_dtypes.bfloat16),
            "wq": wq.astype(ml_dtypes.bfloat16),
            "wk": wk.astype(ml_dtypes.bfloat16),
            "wv": wv.astype(ml_dtypes.bfloat16),
            "w2": w2.astype(ml_dtypes.bfloat16),
            "bq": bq, "bk": bk, "bv": bv.astype(ml_dtypes.bfloat16),
            "ones": ones_row.astype(ml_dtypes.bfloat16),
            "masks": mask_sq, "ident": ident,
        })
    return in_maps


def unshard_outputs(results, b_proj):
    b_proj = np.asarray(b_proj, dtype=np.float32)
    out = np.empty((B, T, C), np.float32)
    for b in range(B):
        out[b] = (results[2 * b]["out"].astype(np.float32)
                  + results[2 * b + 1]["out"].astype(np.float32) + b_proj)
    return out


_CACHED_NC = None


def kernel(x, W_attn, b_attn, W_proj, b_proj):
    global _CACHED_NC
    from concourse import bass_utils

    if _CACHED_NC is None:
        _CACHED_NC = build_program(1, pace=3)
    in_maps = shard_inputs(x, W_attn, b_attn, W_proj, b_proj)
    res = bass_utils.run_bass_kernel_spmd(
        _CACHED_NC, in_maps, core_ids=list(range(N_CORES))
    )
    return unshard_outputs(res.results, b_proj)
